# revision 13
# baseline (speedup 1.0000x reference)
"""Trainium2 Bass kernel for nn_CSFlow (RAFT-style correlation pyramid lookup).

Math restructure (exact up to fp16 rounding):
  - corr(q, pos) = <fmap1[:, q], fmap2[:, pos]> / sqrt(D). Pooling the corr
    volume over (i, j) == pooling fmap2 (linearity), so each pyramid level is
    its own matmul against a pooled fmap2.
  - All 81 lookup offsets of one query share the same fractional bilinear
    weights (integer offsets), so the lookup = gather of a 10x10 integer
    window + separable 2-tap blends with per-query weights.
  - The 10x10 window at a per-query position is fetched from an HBM scratch
    copy of that query's corr maps with ONE indirect-DMA descriptor per
    (query, level): a contiguous band of 9*S+10 elements (S = inner-axis
    size) starting at the window origin. Out-of-range taps are zeroed via
    host-precomputed masks folded into the stage-1 blend weights; band reads
    that spill outside a query's map hit that query's neighbouring level
    sections / the next query / pre-zeroed guards, so they are finite and
    masked.
  - Levels 0-2 store maps x-major (inner = y); level 3 y-major (inner = x,
    size 20) because H_3 = 6 < 10 would make the window view overlap. The
    host permutes level-3 output channels back.

v2 layout/schedule (vs the chunk-wise v1):
  - Scratch is query-major: one [128, NPOS] fp16 staging buffer per tile in
    SBUF, ONE whole-tile DMA to HBM (efficient 20KB descriptors, 12x fewer
    DMA instructions).
  - fp16 matmul inputs (PE fast-weight-load, half the prologue DMA).
  - Blends are software-pipelined `pipe` tiles behind the gathers so the
    DVE/ACT in-order queues never stall on gather completion.
  - No PE transposes: output stays [query_partition, channel] on device and
    the host transposes during assembly.

Sharding: 8 cores x 1920 queries (B*H*W = 15360 split contiguously; cores
0-3 handle batch 0, cores 4-7 batch 1). kernel() takes full inputs and
returns the full output; everything device-side runs SPMD on 8 cores.
"""

import numpy as np

import concourse.bass as bass
import concourse.mybir as mybir
import concourse.tile as tile
from concourse import bacc
from concourse.bass_utils import run_bass_kernel_spmd

# problem shape (hardcoded per harness contract)
B, D, H, W = 2, 256, 48, 160
NCORES = 8
QPC = (B * H * W) // NCORES      # 1920 queries per core
P = 128                          # queries per tile (partitions)
NT = QPC // P                    # 15 tiles per core
NLVL = 4
LH = [48, 24, 12, 6]
LW = [160, 80, 40, 20]
LHW = [LH[i] * LW[i] for i in range(NLVL)]           # 7680 1920 480 120
LOFF = [0, 7680, 9600, 10080]                        # level offset inside a query's map
NPOS = 10200
XMAJ = [True, True, True, False]                     # storage orientation
ST = [48, 24, 12, 20]                                # inner-axis size
BAND = [9 * s + 10 for s in ST]                      # 442 226 118 190
BMAX = 442
HEAD = 512                                           # scratch head/tail guard
SCRN1 = HEAD + P * NPOS + HEAD                       # per-tile scratch elems

F16 = mybir.dt.float16
F32 = mybir.dt.float32
I32 = mybir.dt.int32

PSUM_CHUNK = 1024
MM_CHUNK = 512


def _chunks(total, step):
    return [(o, min(step, total - o)) for o in range(0, total, step)]


def build_nc(repeat=1, do_write=True, do_gather=True, do_blend=True, do_mm=True,
             do_copy=True, do_out=True, copy_pat="vaavavaavava", pipe=2, ps_bufs=4,
             f2_chunks=4, cb_bufs=2, pair=2, mm_chunk=MM_CHUNK, tt2_pool=0):
    nc = bacc.Bacc("TRN2", target_bir_lowering=False, debug=False)

    f1t = nc.dram_tensor("f1t", [2, P, QPC], F16, kind="ExternalInput")
    f2t = nc.dram_tensor("f2t", [2, P, NPOS], F16, kind="ExternalInput")
    idxt = nc.dram_tensor("idxt", [P, NLVL * NT], I32, kind="ExternalInput")
    # stage-2 blend per-query scalars, 2 per (lvl,tile)
    wgtt = nc.dram_tensor("wgtt", [P, NLVL * NT * 2], F32, kind="ExternalInput")
    # stage-1 blend weights with validity masks folded in, 90 per (lvl,tile)
    my0t = nc.dram_tensor("my0t", [P, NLVL * NT * 90], F16, kind="ExternalInput")
    my1t = nc.dram_tensor("my1t", [P, NLVL * NT * 90], F16, kind="ExternalInput")
    outp = nc.dram_tensor("outp", [P, NT * NLVL * 81], F16, kind="ExternalOutput")

    with tile.TileContext(nc) as tc:
        with (
            tc.tile_pool(name="dram", bufs=1, space="DRAM") as dpool,
            tc.tile_pool(name="const", bufs=1) as cpool,
            tc.tile_pool(name="corrbuf", bufs=cb_bufs) as cbpool,
            tc.tile_pool(name="bands", bufs=pipe + 2) as bpool,
            tc.tile_pool(name="blend", bufs=6) as blpool,
            tc.tile_pool(name="otile", bufs=4) as opool,
            tc.tile_pool(name="psum", bufs=ps_bufs, space="PSUM") as pspool,
        ):
            # ---- constants / persistent tiles ----
            scrt = [dpool.tile([SCRN1], F16, name=f"scrt{t}") for t in range(NT)]

            zguard = cpool.tile([1, HEAD], F16)
            nc.vector.memset(zguard[:], 0.0)
            # zero head+tail guards of every scratch so gathers never read
            # uninitialized HBM (intra-tile band spill lands in written
            # neighbour sections; only the outer edges need zeroing).
            for t in range(NT):
                nc.sync.dma_start(scrt[t][0:HEAD].unsqueeze(0), zguard[0:1, :])
                nc.scalar.dma_start(
                    scrt[t][HEAD + P * NPOS : SCRN1].unsqueeze(0), zguard[0:1, :]
                )

            f1sb = cpool.tile([P, 2 * QPC], F16)
            nc.sync.dma_start(f1sb[:, 0:QPC], f1t[0])
            nc.sync.dma_start(f1sb[:, QPC : 2 * QPC], f1t[1])
            f2sb0 = cpool.tile([P, NPOS], F16)
            f2sb1 = cpool.tile([P, NPOS], F16)
            for coff, csz in _chunks(NPOS, -(-NPOS // f2_chunks)):
                nc.sync.dma_start(f2sb0[:, coff : coff + csz], f2t[0][:, coff : coff + csz])
                nc.scalar.dma_start(f2sb1[:, coff : coff + csz], f2t[1][:, coff : coff + csz])
            idx_sb = cpool.tile([P, NLVL * NT], I32)
            nc.sync.dma_start(idx_sb[:], idxt[:])
            wgt_sb = cpool.tile([P, NLVL * NT * 2], F32)
            nc.sync.dma_start(wgt_sb[:], wgtt[:])
            my0_sb = cpool.tile([P, NLVL * NT * 90], F16)
            nc.scalar.dma_start(my0_sb[:], my0t[:])
            my1_sb = cpool.tile([P, NLVL * NT * 90], F16)
            nc.scalar.dma_start(my1_sb[:], my1t[:])

            chunk_list = [
                (l, coff, csz)
                for l in range(NLVL)
                for (coff, csz) in _chunks(LHW[l], PSUM_CHUNK)
            ]

            def blend_stage1(t, band, l, t1):
                # t1 = g0*m0 + g1*m1 (masks fold wy blend + validity)
                s_in = ST[l]
                lo = l * BMAX
                bw = band[:, lo : lo + 10 * s_in].rearrange("p (r s) -> p r s", s=s_in)
                c90 = (l * NT + t) * 90
                m0 = my0_sb[:, c90 : c90 + 90].rearrange("p (r j) -> p r j", j=9)
                m1 = my1_sb[:, c90 : c90 + 90].rearrange("p (r j) -> p r j", j=9)
                t2 = blpool.tile([P, 90], F32, name="t2")
                t1v = t1[:].rearrange("p (r j) -> p r j", j=9)
                t2v = t2[:].rearrange("p (r j) -> p r j", j=9)
                nc.vector.tensor_tensor(
                    out=t1v, in0=bw[:, 0:10, 0:9], in1=m0, op=mybir.AluOpType.mult
                )
                tt2 = nc.gpsimd if tt2_pool else nc.vector
                tt2.tensor_tensor(
                    out=t2v, in0=bw[:, 0:10, 1:10], in1=m1, op=mybir.AluOpType.mult
                )
                nc.gpsimd.tensor_add(out=t1[:], in0=t1[:], in1=t2[:])

            def blend_stage2(t, l, t1, otile):
                # o = t1[0:9]*(1-wx) (ACT), then o = t1[1:10]*wx + o (DVE)
                t1r = t1[:].rearrange("p (r j) -> p r j", j=9)
                c2 = (l * NT + t) * 2
                ov = otile[:, l * 81 : (l + 1) * 81].rearrange("p (a j) -> p a j", j=9)
                nc.scalar.mul(ov, t1r[:, 0:9, :], wgt_sb[:, c2 : c2 + 1])
                nc.vector.scalar_tensor_tensor(
                    out=ov,
                    in0=t1r[:, 1:10, :],
                    scalar=wgt_sb[:, c2 + 1 : c2 + 2],
                    in1=ov,
                    op0=mybir.AluOpType.mult,
                    op1=mybir.AluOpType.add,
                )

            # ---- main loop over query tiles ----
            import contextlib

            rep_ctx = tc.For_i(0, repeat, 1) if repeat > 1 else contextlib.nullcontext()
            with rep_ctx:
                copy_rr = 0
                bands = {}
                groups = [
                    chunk_list[i : i + pair] for i in range(0, len(chunk_list), pair)
                ]
                blend_on = do_blend and do_gather and do_write

                def emit_tile(t, tb):
                    """tb: tile whose blend work interleaves into this tile's
                    copy stream (None for none). Level l's stage-1 is emitted
                    after chunk-group l, its stage-2 one group later, so each
                    engine sees at most a couple of small blend ops between
                    psum-draining copies and every cross-engine dependency has
                    a full group of slack."""
                    nonlocal copy_rr
                    cb = cbpool.tile([P, NPOS], F16, name="cb")
                    t1s = {}
                    otile = opool.tile([P, NLVL * 81], F16, name="otile") if tb is not None else None
                    for gi, grp in enumerate(groups):
                        pss = [
                            pspool.tile([P, PSUM_CHUNK], F32, name="cps")[:, :csz]
                            for (_, _, csz) in grp
                        ]
                        for k in range(2 if do_mm else 0):
                            f2sb = f2sb0 if k == 0 else f2sb1
                            wgt = f1sb[:, k * QPC + t * P : k * QPC + (t + 1) * P]
                            for ps, (l, coff, csz) in zip(pss, grp):
                                for soff, ssz in _chunks(csz, mm_chunk):
                                    nc.tensor.matmul(
                                        ps[:, soff : soff + ssz],
                                        wgt,
                                        f2sb[
                                            :,
                                            LOFF[l] + coff + soff : LOFF[l]
                                            + coff
                                            + soff
                                            + ssz,
                                        ],
                                        start=(k == 0),
                                        stop=(k == 1),
                                        skip_group_check=(pair > 1),
                                    )
                        if do_mm and do_copy:
                            for ps, (l, coff, csz) in zip(pss, grp):
                                dst = cb[:, LOFF[l] + coff : LOFF[l] + coff + csz]
                                eng = copy_pat[copy_rr % len(copy_pat)]
                                if eng == "v":
                                    nc.vector.tensor_copy(dst, ps)
                                else:
                                    nc.scalar.copy(dst, ps)
                                copy_rr += 1
                        if tb is not None:
                            if gi < NLVL:
                                t1s[gi] = blpool.tile([P, 90], F32, name="t1")
                                blend_stage1(tb, bands[tb], gi, t1s[gi])
                            if 1 <= gi <= NLVL:
                                blend_stage2(tb, gi - 1, t1s[gi - 1], otile)
                    if tb is not None:
                        bands.pop(tb)
                        if do_out:
                            nc.sync.dma_start(
                                outp[:, tb * NLVL * 81 : (tb + 1) * NLVL * 81],
                                otile[:],
                            )
                    # === one whole-tile write, then 4 gathers ===
                    if do_mm and do_copy and do_write:
                        nc.sync.dma_start(
                            scrt[t][HEAD : HEAD + P * NPOS].rearrange(
                                "(p x) -> p x", x=NPOS
                            ),
                            cb[:],
                        )
                    if do_gather and do_write:
                        band = bpool.tile([P, NLVL * BMAX], F16, name="band")
                        for l in range(NLVL):
                            nc.gpsimd.indirect_dma_start(
                                out=band[:, l * BMAX : l * BMAX + BAND[l]],
                                out_offset=None,
                                in_=scrt[t][:].unsqueeze(1),
                                in_offset=bass.IndirectOffsetOnAxis(
                                    ap=idx_sb[:, t * NLVL + l : t * NLVL + l + 1],
                                    axis=0,
                                ),
                                element_offset=0,
                            )
                        bands[t] = band

                def emit_blend_only(tb):
                    otile = opool.tile([P, NLVL * 81], F16, name="otile")
                    t1s = {}
                    for l in range(NLVL):
                        t1s[l] = blpool.tile([P, 90], F32, name="t1")
                        blend_stage1(tb, bands[tb], l, t1s[l])
                        if l >= 1:
                            blend_stage2(tb, l - 1, t1s[l - 1], otile)
                    blend_stage2(tb, NLVL - 1, t1s[NLVL - 1], otile)
                    bands.pop(tb)
                    if do_out:
                        nc.sync.dma_start(
                            outp[:, tb * NLVL * 81 : (tb + 1) * NLVL * 81], otile[:]
                        )

                for t in range(NT):
                    emit_tile(t, t - pipe if (blend_on and t >= pipe) else None)
                if blend_on:
                    for t in range(max(0, NT - pipe), NT):
                        emit_blend_only(t)

    nc.compile()
    return nc


# ---------------- host side ----------------

def _pool2(x):
    n, c, h, w = x.shape
    return x.reshape(n, c, h // 2, 2, w // 2, 2).mean(axis=(3, 5))


def _host_prep(fmap1, fmap2, coords):
    fmap1 = np.asarray(fmap1, np.float32)
    fmap2 = np.asarray(fmap2, np.float32)
    coords = np.asarray(coords, np.float32)
    scale = np.float32(1.0 / np.sqrt(D))

    # pooled fmap2 levels, flattened in storage orientation, scaled
    levels = []
    cur = fmap2 * scale
    for l in range(NLVL):
        if XMAJ[l]:
            levels.append(
                np.ascontiguousarray(cur.transpose(0, 1, 3, 2)).reshape(B, D, LHW[l])
            )
        else:
            levels.append(cur.reshape(B, D, LHW[l]))
        if l < NLVL - 1:
            cur = _pool2(cur)
    f2cat = np.concatenate(levels, axis=2).astype(np.float16)  # [B, D, NPOS]

    cx = coords[:, 0].reshape(-1)  # [B*H*W], query q = b*H*W + h*W + w
    cy = coords[:, 1].reshape(-1)
    nq = cx.shape[0]

    idx_all = np.zeros((NLVL, nq), np.int32)
    wgt_all = np.zeros((NLVL, nq, 2), np.float32)
    my0_all = np.zeros((NLVL, nq, 10, 9), np.float16)
    my1_all = np.zeros((NLVL, nq, 10, 9), np.float16)
    q_tile = (np.arange(nq) % P).astype(np.int64)  # partition within tile
    rr = np.arange(10)
    for l in range(NLVL):
        inv = np.float32(1.0 / (1 << l))
        x = cx * inv
        y = cy * inv
        x0 = np.floor(x)
        y0 = np.floor(y)
        wx = (x - x0).astype(np.float32)
        wy = (y - y0).astype(np.float32)
        x0c = np.clip(x0, -5, LW[l] + 4).astype(np.int64)
        y0c = np.clip(y0, -5, LH[l] + 4).astype(np.int64)
        vx = ((x0[:, None] + rr[None, :] - 4) >= 0) & (
            (x0[:, None] + rr[None, :] - 4) <= LW[l] - 1
        )  # [nq, 10] validity of x-tap x0-4+i
        vy = ((y0[:, None] + rr[None, :] - 4) >= 0) & (
            (y0[:, None] + rr[None, :] - 4) <= LH[l] - 1
        )
        if XMAJ[l]:
            # outer = x (weight wx), inner = y (weight wy)
            idx_all[l] = (
                HEAD + q_tile * NPOS + LOFF[l] + (x0c - 4) * LH[l] + (y0c - 4)
            ).astype(np.int32)
            wgt_all[l, :, 0] = 1.0 - wx
            wgt_all[l, :, 1] = wx
            m0 = vx[:, :, None] & vy[:, None, 0:9]
            m1 = vx[:, :, None] & vy[:, None, 1:10]
            my0_all[l] = m0 * (1.0 - wy)[:, None, None]
            my1_all[l] = m1 * wy[:, None, None]
        else:
            # outer = y (weight wy), inner = x (weight wx)
            idx_all[l] = (
                HEAD + q_tile * NPOS + LOFF[l] + (y0c - 4) * LW[l] + (x0c - 4)
            ).astype(np.int32)
            wgt_all[l, :, 0] = 1.0 - wy
            wgt_all[l, :, 1] = wy
            m0 = vy[:, :, None] & vx[:, None, 0:9]
            m1 = vy[:, :, None] & vx[:, None, 1:10]
            my0_all[l] = m0 * (1.0 - wx)[:, None, None]
            my1_all[l] = m1 * wx[:, None, None]

    f1r = fmap1.reshape(B, D, H * W).astype(np.float16)

    def core_map(c):
        b = c // (NCORES // B)
        cl = c % (NCORES // B)
        sl = slice(c * QPC, (c + 1) * QPC)
        f1c = f1r[b][:, cl * QPC : (cl + 1) * QPC]
        return {
            "f1t": np.ascontiguousarray(f1c.reshape(2, P, QPC)),
            "f2t": np.ascontiguousarray(f2cat[b].reshape(2, P, NPOS)),
            "idxt": np.ascontiguousarray(
                idx_all[:, sl].reshape(NLVL, NT, P).transpose(2, 1, 0).reshape(P, -1)
            ),
            "wgtt": np.ascontiguousarray(
                wgt_all[:, sl].reshape(NLVL, NT, P, 2)
                .transpose(2, 0, 1, 3)
                .reshape(P, -1)
            ),
            "my0t": np.ascontiguousarray(
                my0_all[:, sl].reshape(NLVL, NT, P, 90)
                .transpose(2, 0, 1, 3)
                .reshape(P, -1)
            ),
            "my1t": np.ascontiguousarray(
                my1_all[:, sl].reshape(NLVL, NT, P, 90)
                .transpose(2, 0, 1, 3)
                .reshape(P, -1)
            ),
        }

    return [core_map(c) for c in range(NCORES)]


def assemble(results):
    out = np.empty((B, NLVL * 81, H * W), np.float32)
    for c in range(NCORES):
        b = c // (NCORES // B)
        lo = (c % (NCORES // B)) * QPC
        r = np.asarray(results[c]["outp"], np.float32).reshape(P, NT, NLVL, 81)
        r = r.transpose(2, 3, 1, 0).reshape(NLVL, 81, QPC)  # query = t*P + p
        for l in range(NLVL):
            blk = r[l]
            if not XMAJ[l]:
                # stored channel order is bi*9+a; reference wants 9a+bi
                blk = blk.reshape(9, 9, QPC).transpose(1, 0, 2).reshape(81, QPC)
            out[b, l * 81 : (l + 1) * 81, lo : lo + QPC] = blk
    return out.reshape(B, NLVL * 81, H, W)


_NC_CACHE = {}


def get_nc():
    if "nc" not in _NC_CACHE:
        _NC_CACHE["nc"] = build_nc()
    return _NC_CACHE["nc"]


def kernel(fmap1, fmap2, coords):
    in_maps = _host_prep(fmap1, fmap2, coords)
    nc = get_nc()
    res = run_bass_kernel_spmd(nc, in_maps, core_ids=list(range(NCORES)))
    return assemble(res.results)


# revision 33
# speedup vs baseline: 1.5428x; 1.5428x over previous
"""Trainium2 Bass kernel for nn_CSFlow (RAFT-style correlation pyramid lookup).

Math restructure (exact up to fp16 rounding):
  - corr(q, pos) = <fmap1[:, q], fmap2[:, pos]> / sqrt(D). Pooling the corr
    volume over (i, j) == pooling fmap2 (linearity), so each pyramid level is
    its own matmul against a pooled fmap2.
  - All 81 lookup offsets of one query share the same fractional bilinear
    weights (integer offsets), so the lookup = gather of a 10x10 integer
    window + separable 2-tap blends with per-query weights.
  - The 10x10 window at a per-query position is fetched from an HBM scratch
    copy of that query's corr maps with ONE indirect-DMA descriptor per
    (query, level): a contiguous band of 9*S+10 elements (S = inner-axis
    size) starting at the window origin. Out-of-range taps are zeroed via
    host-precomputed masks folded into the stage-1 blend weights; band reads
    that spill outside a query's map hit that query's neighbouring level
    sections / the next query / pre-zeroed guards, so they are finite and
    masked.
  - Levels 0-2 store maps x-major (inner = y); level 3 y-major (inner = x,
    size 20) because H_3 = 6 < 10 would make the window view overlap. The
    host permutes level-3 output channels back.

v2 layout/schedule (vs the chunk-wise v1):
  - Scratch is query-major: one [128, NPOS] fp16 staging buffer per tile in
    SBUF, ONE whole-tile DMA to HBM (efficient 20KB descriptors, 12x fewer
    DMA instructions).
  - fp16 matmul inputs (PE fast-weight-load, half the prologue DMA).
  - Blends are software-pipelined `pipe` tiles behind the gathers so the
    DVE/ACT in-order queues never stall on gather completion.
  - No PE transposes: output stays [query_partition, channel] on device and
    the host transposes during assembly.

Sharding: 8 cores x 1920 queries (B*H*W = 15360 split contiguously; cores
0-3 handle batch 0, cores 4-7 batch 1). kernel() takes full inputs and
returns the full output; everything device-side runs SPMD on 8 cores.
"""

import numpy as np

import concourse.bass as bass
import concourse.mybir as mybir
import concourse.tile as tile
from concourse import bacc
from concourse.bass_utils import run_bass_kernel_spmd

# problem shape (hardcoded per harness contract)
B, D, H, W = 2, 256, 48, 160
NCORES = 8
QPC = (B * H * W) // NCORES      # 1920 queries per core
P = 128                          # queries per tile (partitions)
NT = QPC // P                    # 15 tiles per core
NLVL = 4
LH = [48, 24, 12, 6]
LW = [160, 80, 40, 20]
LHW = [LH[i] * LW[i] for i in range(NLVL)]           # 7680 1920 480 120
LOFF = [0, 7680, 9600, 10080]                        # level offset inside a query's map
NPOS = 10200
XMAJ = [True, True, True, False]                     # storage orientation
ST = [48, 24, 12, 20]                                # inner-axis size
BAND = [9 * s + 10 for s in ST]                      # 442 226 118 190
BMAX = 442
HEAD = 512                                           # scratch head/tail guard
SCRN1 = HEAD + P * NPOS + HEAD                       # per-tile scratch elems

F16 = mybir.dt.float16
F32 = mybir.dt.float32
I32 = mybir.dt.int32

PSUM_CHUNK = 1024
MM_CHUNK = 512

# per-level storage: [outer O, inner I]; levels 0-2 x-major (inner=y),
# level 3 y-major (inner=x)
INNER = [48, 24, 12, 20]
OUTER = [160, 80, 40, 6]


def _chunks(total, step):
    return [(o, min(step, total - o)) for o in range(0, total, step)]


NCPB = NCORES // B      # cores per batch
WC = W // NCPB          # w-columns per core (w-split sharding)


def _geometry(cy, tt_q):
    """Per-tile (inner_start, inner_cnt) per level, valid across all 8 cores.

    Levels 0/1 restrict the stored/computed y-rows to the union of the
    tile's query windows (coords are host-known); levels 2/3 stay full.
    With w-split sharding every core's tile t covers the same h-rows, so
    the 8-core union stays narrow. Invalid-y window taps outside the true
    map read neighbouring columns and are zeroed by the blend masks."""
    geo = []
    for t in range(NT):
        sel = tt_q == t
        ent = []
        for l in range(NLVL):
            if l >= 2:
                ent.append((0, INNER[l]))
                continue
            y0c = np.clip(np.floor(cy[sel] * (1.0 / (1 << l))), -5, LH[l] + 4)
            ys = max(int(y0c.min()) - 4, 0)
            ye = min(int(y0c.max()) + 5, LH[l] - 1)
            ent.append((ys, ye - ys + 1))
        geo.append(tuple(ent))
    return tuple(geo)


def _full_geometry():
    return tuple(tuple((0, INNER[l]) for l in range(NLVL)) for _ in range(NT))


def _tile_layout(g):
    """loff[l] inside a query's map, npos, band slot offsets, gather lens.

    Band slots are 10*cnt wide (gather fills 9*cnt+10 of that) so the
    blend's [10, cnt] strided view stays inside its slot."""
    loff, off = [], 0
    for l in range(NLVL):
        loff.append(off)
        off += OUTER[l] * g[l][1]
    bsl = [9 * g[l][1] + 10 for l in range(NLVL)]
    boff_v, o2 = [], 0
    for l in range(NLVL):
        boff_v.append(o2)
        o2 += 10 * g[l][1]
    return loff, off, boff_v, bsl


def _local_order():
    """Per-global-query local index q' = h*WC + (w % WC) and helpers."""
    g = np.arange(B * H * W)
    rr_ = g % (H * W)
    ww = rr_ % W
    qp = (rr_ // W) * WC + (ww % WC)
    return qp, (qp % P).astype(np.int64), (qp // P).astype(np.int64)


def _core_ids(c):
    """Global query ids of core c, in local q' order."""
    b, cl = c // NCPB, c % NCPB
    return (
        b * H * W
        + (np.arange(H)[:, None] * W + cl * WC + np.arange(WC)[None, :]).ravel()
    )


def _tile_chunks(g, mm_chunk):
    """psum chunks: (l, cboff, csz, [(ooff, osz), ...]) — up to 2 matmul
    units of ≤mm_chunk cols each, same level, contiguous in the staging
    buffer."""
    loff, _, _, _ = _tile_layout(g)
    out = []
    for l in range(NLVL):
        cnt = g[l][1]
        no = min(OUTER[l], max(1, mm_chunk // cnt))
        units = _chunks(OUTER[l], no)
        for i in range(0, len(units), 2):
            pairu = units[i : i + 2]
            csz = sum(osz * cnt for (_, osz) in pairu)
            out.append((l, loff[l] + pairu[0][0] * cnt, csz, pairu))
    return out


def build_nc(repeat=1, do_write=True, do_gather=True, do_blend=True, do_mm=True,
             do_copy=True, do_out=True, copy_pat="vaavavaavava", pipe=2, ps_bufs=4,
             f2_chunks=4, cb_bufs=2, pair=2, mm_chunk=MM_CHUNK, tt2_pool=0, gmerge=0,
             geo=None):
    if geo is None:
        geo = _GEO_CACHE.get("geo") or _full_geometry()
    layouts = [_tile_layout(g) for g in geo]        # (loff, npos, boff, bsl)
    tchunks = [_tile_chunks(g, mm_chunk) for g in geo]
    npos_max = max(lay[1] for lay in layouts)
    btot_max = max(
        lay[2][-1] + 10 * g[-1][1] for lay, g in zip(layouts, geo)
    )
    nc = bacc.Bacc("TRN2", target_bir_lowering=False, debug=False)

    f1t = nc.dram_tensor("f1t", [2, P, QPC], F16, kind="ExternalInput")
    f2t = nc.dram_tensor("f2t", [2, P, NPOS], F16, kind="ExternalInput")
    idxt = nc.dram_tensor("idxt", [P, NLVL * NT], I32, kind="ExternalInput")
    # stage-2 blend per-query scalars, 2 per (lvl,tile)
    wgtt = nc.dram_tensor("wgtt", [P, NLVL * NT * 2], F32, kind="ExternalInput")
    # stage-1 blend weights with validity masks folded in, 90 per (lvl,tile)
    my0t = nc.dram_tensor("my0t", [P, NLVL * NT * 90], F16, kind="ExternalInput")
    my1t = nc.dram_tensor("my1t", [P, NLVL * NT * 90], F16, kind="ExternalInput")
    outp = nc.dram_tensor("outp", [P, NT * NLVL * 81], F16, kind="ExternalOutput")

    with tile.TileContext(nc) as tc:
        with (
            tc.tile_pool(name="dram", bufs=1, space="DRAM") as dpool,
            tc.tile_pool(name="const", bufs=1) as cpool,
            tc.tile_pool(name="corrbuf", bufs=cb_bufs) as cbpool,
            tc.tile_pool(name="bands", bufs=pipe + 2) as bpool,
            tc.tile_pool(name="blend", bufs=6) as blpool,
            tc.tile_pool(name="otile", bufs=4) as opool,
            tc.tile_pool(name="psum", bufs=ps_bufs, space="PSUM") as pspool,
        ):
            # ---- constants / persistent tiles ----
            scrt = [
                dpool.tile([HEAD + P * layouts[t][1] + HEAD], F16, name=f"scrt{t}")
                for t in range(NT)
            ]

            zguard = cpool.tile([1, HEAD], F16)
            nc.vector.memset(zguard[:], 0.0)
            # zero head+tail guards of every scratch so gathers never read
            # uninitialized HBM (intra-tile band spill lands in written
            # neighbour sections; only the outer edges need zeroing).
            for t in range(NT):
                npos_t = layouts[t][1]
                nc.sync.dma_start(scrt[t][0:HEAD].unsqueeze(0), zguard[0:1, :])
                nc.scalar.dma_start(
                    scrt[t][HEAD + P * npos_t : HEAD + P * npos_t + HEAD].unsqueeze(0),
                    zguard[0:1, :],
                )

            f1sb = cpool.tile([P, 2 * QPC], F16)
            nc.sync.dma_start(f1sb[:, 0:QPC], f1t[0])
            nc.sync.dma_start(f1sb[:, QPC : 2 * QPC], f1t[1])
            f2sb0 = cpool.tile([P, NPOS], F16)
            f2sb1 = cpool.tile([P, NPOS], F16)
            for coff, csz in _chunks(NPOS, -(-NPOS // f2_chunks)):
                nc.sync.dma_start(f2sb0[:, coff : coff + csz], f2t[0][:, coff : coff + csz])
                nc.scalar.dma_start(f2sb1[:, coff : coff + csz], f2t[1][:, coff : coff + csz])
            idx_sb = cpool.tile([P, NLVL * NT], I32)
            nc.sync.dma_start(idx_sb[:], idxt[:])
            wgt_sb = cpool.tile([P, NLVL * NT * 2], F32)
            nc.sync.dma_start(wgt_sb[:], wgtt[:])
            my0_sb = cpool.tile([P, NLVL * NT * 90], F16)
            nc.scalar.dma_start(my0_sb[:], my0t[:])
            my1_sb = cpool.tile([P, NLVL * NT * 90], F16)
            nc.scalar.dma_start(my1_sb[:], my1t[:])

            # [P, outer, inner] views of the two f2 halves, one per level
            f2v0 = [
                f2sb0[:, LOFF[l] : LOFF[l] + LHW[l]].rearrange(
                    "p (o i) -> p o i", i=INNER[l]
                )
                for l in range(NLVL)
            ]
            f2v1 = [
                f2sb1[:, LOFF[l] : LOFF[l] + LHW[l]].rearrange(
                    "p (o i) -> p o i", i=INNER[l]
                )
                for l in range(NLVL)
            ]

            def blend_stage1(t, band, l, t1):
                # t1 = g0*m0 + g1*m1 (masks fold wy blend + validity)
                s_in = geo[t][l][1]
                lo = layouts[t][2][l]
                bw = band[:, lo : lo + 10 * s_in].rearrange("p (r s) -> p r s", s=s_in)
                c90 = (l * NT + t) * 90
                m0 = my0_sb[:, c90 : c90 + 90].rearrange("p (r j) -> p r j", j=9)
                m1 = my1_sb[:, c90 : c90 + 90].rearrange("p (r j) -> p r j", j=9)
                t2 = blpool.tile([P, 90], F32, name="t2")
                t1v = t1[:].rearrange("p (r j) -> p r j", j=9)
                t2v = t2[:].rearrange("p (r j) -> p r j", j=9)
                nc.vector.tensor_tensor(
                    out=t1v, in0=bw[:, 0:10, 0:9], in1=m0, op=mybir.AluOpType.mult
                )
                tt2 = nc.gpsimd if tt2_pool else nc.vector
                tt2.tensor_tensor(
                    out=t2v, in0=bw[:, 0:10, 1:10], in1=m1, op=mybir.AluOpType.mult
                )
                nc.gpsimd.tensor_add(out=t1[:], in0=t1[:], in1=t2[:])

            def blend_stage2(t, l, t1, otile):
                # o = t1[0:9]*(1-wx) (ACT), then o = t1[1:10]*wx + o (DVE)
                t1r = t1[:].rearrange("p (r j) -> p r j", j=9)
                c2 = (l * NT + t) * 2
                ov = otile[:, l * 81 : (l + 1) * 81].rearrange("p (a j) -> p a j", j=9)
                nc.scalar.mul(ov, t1r[:, 0:9, :], wgt_sb[:, c2 : c2 + 1])
                nc.vector.scalar_tensor_tensor(
                    out=ov,
                    in0=t1r[:, 1:10, :],
                    scalar=wgt_sb[:, c2 + 1 : c2 + 2],
                    in1=ov,
                    op0=mybir.AluOpType.mult,
                    op1=mybir.AluOpType.add,
                )

            # ---- main loop over query tiles ----
            import contextlib

            rep_ctx = tc.For_i(0, repeat, 1) if repeat > 1 else contextlib.nullcontext()
            with rep_ctx:
                copy_rr = 0
                bands = {}
                blend_on = do_blend and do_gather and do_write

                def emit_tile(t, tb):
                    """tb: tile whose blend work interleaves into this tile's
                    copy stream (None for none). Level l's stage-1 is emitted
                    after chunk-group l, its stage-2 one group later, so each
                    engine sees at most a couple of small blend ops between
                    psum-draining copies and every cross-engine dependency has
                    a full group of slack."""
                    nonlocal copy_rr
                    npos_t = layouts[t][1]
                    cb = cbpool.tile([P, npos_max], F16, name="cb")
                    groups = [
                        tchunks[t][i : i + pair]
                        for i in range(0, len(tchunks[t]), pair)
                    ]
                    t1s = {}
                    otile = opool.tile([P, NLVL * 81], F16, name="otile") if tb is not None else None
                    for gi, grp in enumerate(groups):
                        pss = [
                            pspool.tile([P, PSUM_CHUNK], F32, name="cps")
                            for _ in grp
                        ]
                        for k in range(2 if do_mm else 0):
                            f2v = f2v0 if k == 0 else f2v1
                            wgt = f1sb[:, k * QPC + t * P : k * QPC + (t + 1) * P]
                            for ps, (l, cboff, csz, units) in zip(pss, grp):
                                s, cnt = geo[t][l]
                                # each unit starts on a PSUM bank boundary: a
                                # matmul's output must not cross a bank
                                for ui, (ooff, osz) in enumerate(units):
                                    nc.tensor.matmul(
                                        ps[:, ui * 512 : ui * 512 + osz * cnt],
                                        wgt,
                                        f2v[l][:, ooff : ooff + osz, s : s + cnt],
                                        start=(k == 0),
                                        stop=(k == 1),
                                        skip_group_check=(pair > 1),
                                    )
                        if do_mm and do_copy:
                            for ps, (l, cboff, csz, units) in zip(pss, grp):
                                s, cnt = geo[t][l]
                                loff_l = layouts[t][0][l]
                                for ui, (ooff, osz) in enumerate(units):
                                    dst = cb[
                                        :,
                                        loff_l + ooff * cnt : loff_l + (ooff + osz) * cnt,
                                    ]
                                    eng = copy_pat[copy_rr % len(copy_pat)]
                                    if eng == "v":
                                        nc.vector.tensor_copy(
                                            dst, ps[:, ui * 512 : ui * 512 + osz * cnt]
                                        )
                                    else:
                                        nc.scalar.copy(
                                            dst, ps[:, ui * 512 : ui * 512 + osz * cnt]
                                        )
                                    copy_rr += 1
                        if tb is not None:
                            if gi < NLVL:
                                t1s[gi] = blpool.tile([P, 90], F32, name="t1")
                                blend_stage1(tb, bands[tb], gi, t1s[gi])
                            if 1 <= gi <= NLVL:
                                blend_stage2(tb, gi - 1, t1s[gi - 1], otile)
                    if tb is not None:
                        # small tiles have fewer chunk-groups than blend slots;
                        # emit whatever didn't fit into the interleave
                        ng = len(groups)
                        for l in range(min(ng, NLVL), NLVL):
                            t1s[l] = blpool.tile([P, 90], F32, name="t1")
                            blend_stage1(tb, bands[tb], l, t1s[l])
                        for l in range(max(min(ng - 1, NLVL), 0), NLVL):
                            blend_stage2(tb, l, t1s[l], otile)
                        bands.pop(tb)
                        if do_out:
                            nc.sync.dma_start(
                                outp[:, tb * NLVL * 81 : (tb + 1) * NLVL * 81],
                                otile[:],
                            )
                    # === one whole-tile write, then 4 gathers ===
                    if do_mm and do_copy and do_write:
                        nc.sync.dma_start(
                            scrt[t][HEAD : HEAD + P * npos_t].rearrange(
                                "(p x) -> p x", x=npos_t
                            ),
                            cb[:, :npos_t],
                        )
                    if do_gather and do_write:
                        band = bpool.tile([P, btot_max], F16, name="band")
                        for l in range(NLVL):
                            boff_t, bsl_t = layouts[t][2], layouts[t][3]
                            nc.gpsimd.indirect_dma_start(
                                out=band[:, boff_t[l] : boff_t[l] + bsl_t[l]],
                                out_offset=None,
                                in_=scrt[t][:].unsqueeze(1),
                                in_offset=bass.IndirectOffsetOnAxis(
                                    ap=idx_sb[:, t * NLVL + l : t * NLVL + l + 1],
                                    axis=0,
                                ),
                                element_offset=0,
                            )
                        bands[t] = band

                def emit_blend_only(tb):
                    otile = opool.tile([P, NLVL * 81], F16, name="otile")
                    t1s = {}
                    for l in range(NLVL):
                        t1s[l] = blpool.tile([P, 90], F32, name="t1")
                        blend_stage1(tb, bands[tb], l, t1s[l])
                        if l >= 1:
                            blend_stage2(tb, l - 1, t1s[l - 1], otile)
                    blend_stage2(tb, NLVL - 1, t1s[NLVL - 1], otile)
                    bands.pop(tb)
                    if do_out:
                        nc.sync.dma_start(
                            outp[:, tb * NLVL * 81 : (tb + 1) * NLVL * 81], otile[:]
                        )

                for t in range(NT):
                    emit_tile(t, t - pipe if (blend_on and t >= pipe) else None)
                if blend_on:
                    for t in range(max(0, NT - pipe), NT):
                        emit_blend_only(t)

    nc.compile()
    return nc


# ---------------- host side ----------------

def _pool2(x):
    n, c, h, w = x.shape
    return x.reshape(n, c, h // 2, 2, w // 2, 2).mean(axis=(3, 5))


def _host_prep(fmap1, fmap2, coords):
    fmap1 = np.asarray(fmap1, np.float32)
    fmap2 = np.asarray(fmap2, np.float32)
    coords = np.asarray(coords, np.float32)
    scale = np.float32(1.0 / np.sqrt(D))

    # pooled fmap2 levels, flattened in storage orientation, scaled
    levels = []
    cur = fmap2 * scale
    for l in range(NLVL):
        if XMAJ[l]:
            levels.append(
                np.ascontiguousarray(cur.transpose(0, 1, 3, 2)).reshape(B, D, LHW[l])
            )
        else:
            levels.append(cur.reshape(B, D, LHW[l]))
        if l < NLVL - 1:
            cur = _pool2(cur)
    f2cat = np.concatenate(levels, axis=2).astype(np.float16)  # [B, D, NPOS]

    cx = coords[:, 0].reshape(-1)  # [B*H*W], query q = b*H*W + h*W + w
    cy = coords[:, 1].reshape(-1)
    nq = cx.shape[0]

    # w-split sharding: core c takes w in [cl*WC, (cl+1)*WC) of every row,
    # so tile t covers the same h-rows on every core and the per-tile
    # y-window restriction survives the 8-core union.
    _, q_tile, tt_q = _local_order()
    geo = _geometry(cy, tt_q)
    _GEO_CACHE["geo"] = geo
    layouts = [_tile_layout(g) for g in geo]
    npos_q = np.array([layouts[t][1] for t in range(NT)], np.int64)[tt_q]

    idx_all = np.zeros((NLVL, nq), np.int32)
    wgt_all = np.zeros((NLVL, nq, 2), np.float32)
    my0_all = np.zeros((NLVL, nq, 10, 9), np.float16)
    my1_all = np.zeros((NLVL, nq, 10, 9), np.float16)
    rr = np.arange(10)
    for l in range(NLVL):
        inv = np.float32(1.0 / (1 << l))
        x = cx * inv
        y = cy * inv
        x0 = np.floor(x)
        y0 = np.floor(y)
        wx = (x - x0).astype(np.float32)
        wy = (y - y0).astype(np.float32)
        x0c = np.clip(x0, -5, LW[l] + 4).astype(np.int64)
        y0c = np.clip(y0, -5, LH[l] + 4).astype(np.int64)
        vx = ((x0[:, None] + rr[None, :] - 4) >= 0) & (
            (x0[:, None] + rr[None, :] - 4) <= LW[l] - 1
        )  # [nq, 10] validity of x-tap x0-4+i
        vy = ((y0[:, None] + rr[None, :] - 4) >= 0) & (
            (y0[:, None] + rr[None, :] - 4) <= LH[l] - 1
        )
        s_q = np.array([geo[t][l][0] for t in range(NT)], np.int64)[tt_q]
        cnt_q = np.array([geo[t][l][1] for t in range(NT)], np.int64)[tt_q]
        loff_q = np.array([layouts[t][0][l] for t in range(NT)], np.int64)[tt_q]
        if XMAJ[l]:
            # outer = x (weight wx), inner = y (weight wy)
            idx_all[l] = (
                HEAD + q_tile * npos_q + loff_q + (x0c - 4) * cnt_q + (y0c - 4 - s_q)
            ).astype(np.int32)
            wgt_all[l, :, 0] = 1.0 - wx
            wgt_all[l, :, 1] = wx
            m0 = vx[:, :, None] & vy[:, None, 0:9]
            m1 = vx[:, :, None] & vy[:, None, 1:10]
            my0_all[l] = m0 * (1.0 - wy)[:, None, None]
            my1_all[l] = m1 * wy[:, None, None]
        else:
            # outer = y (weight wy), inner = x (weight wx)
            idx_all[l] = (
                HEAD + q_tile * npos_q + loff_q + (y0c - 4) * cnt_q + (x0c - 4)
            ).astype(np.int32)
            wgt_all[l, :, 0] = 1.0 - wy
            wgt_all[l, :, 1] = wy
            m0 = vy[:, :, None] & vx[:, None, 0:9]
            m1 = vy[:, :, None] & vx[:, None, 1:10]
            my0_all[l] = m0 * (1.0 - wx)[:, None, None]
            my1_all[l] = m1 * wx[:, None, None]

    f1r = fmap1.reshape(B, D, H * W).astype(np.float16)

    def core_map(c):
        b = c // NCPB
        ids = _core_ids(c)
        idl = ids - b * H * W
        f1c = f1r[b][:, idl]
        return {
            "f1t": np.ascontiguousarray(f1c.reshape(2, P, QPC)),
            "f2t": np.ascontiguousarray(f2cat[b].reshape(2, P, NPOS)),
            "idxt": np.ascontiguousarray(
                idx_all[:, ids].reshape(NLVL, NT, P).transpose(2, 1, 0).reshape(P, -1)
            ),
            "wgtt": np.ascontiguousarray(
                wgt_all[:, ids].reshape(NLVL, NT, P, 2)
                .transpose(2, 0, 1, 3)
                .reshape(P, -1)
            ),
            "my0t": np.ascontiguousarray(
                my0_all[:, ids].reshape(NLVL, NT, P, 90)
                .transpose(2, 0, 1, 3)
                .reshape(P, -1)
            ),
            "my1t": np.ascontiguousarray(
                my1_all[:, ids].reshape(NLVL, NT, P, 90)
                .transpose(2, 0, 1, 3)
                .reshape(P, -1)
            ),
        }

    return [core_map(c) for c in range(NCORES)]


def assemble(results):
    out = np.empty((B, NLVL * 81, H * W), np.float32)
    for c in range(NCORES):
        b = c // NCPB
        idl = _core_ids(c) - b * H * W
        r = np.asarray(results[c]["outp"], np.float32).reshape(P, NT, NLVL, 81)
        r = r.transpose(2, 3, 1, 0).reshape(NLVL, 81, QPC)  # local query = t*P + p
        outb = out[b]
        for l in range(NLVL):
            blk = r[l]
            if not XMAJ[l]:
                # stored channel order is bi*9+a; reference wants 9a+bi
                blk = blk.reshape(9, 9, QPC).transpose(1, 0, 2).reshape(81, QPC)
            outb[l * 81 : (l + 1) * 81, idl] = blk
    return out.reshape(B, NLVL * 81, H, W)


_NC_CACHE = {}
_GEO_CACHE = {}


def get_nc():
    key = _GEO_CACHE.get("geo")
    if _NC_CACHE.get("key", 0) != key:
        _NC_CACHE["nc"] = build_nc()
        _NC_CACHE["key"] = key
    return _NC_CACHE["nc"]


def kernel(fmap1, fmap2, coords):
    in_maps = _host_prep(fmap1, fmap2, coords)
    nc = get_nc()
    res = run_bass_kernel_spmd(nc, in_maps, core_ids=list(range(NCORES)))
    return assemble(res.results)


# revision 50
# speedup vs baseline: 2.1055x; 1.3647x over previous
"""Trainium2 Bass kernel for nn_CSFlow (RAFT-style correlation pyramid lookup).

Math restructure (exact up to fp16 rounding):
  - corr(q, pos) = <fmap1[:, q], fmap2[:, pos]> / sqrt(D). Pooling the corr
    volume over (i, j) == pooling fmap2 (linearity), so each pyramid level is
    its own matmul against a pooled fmap2.
  - All 81 lookup offsets of one query share the same fractional bilinear
    weights (integer offsets), so the lookup = gather of a 10x10 integer
    window + separable 2-tap blends with per-query weights.
  - The 10x10 window at a per-query position is fetched from an HBM scratch
    copy of that query's corr maps with ONE indirect-DMA descriptor per
    (query, level): a contiguous band of 9*S+10 elements (S = inner-axis
    size) starting at the window origin. Out-of-range taps are zeroed via
    host-precomputed masks folded into the stage-1 blend weights; band reads
    that spill outside a query's map hit that query's neighbouring level
    sections / the next query / pre-zeroed guards, so they are finite and
    masked.
  - Levels 0-2 store maps x-major (inner = y); level 3 y-major (inner = x,
    size 20) because H_3 = 6 < 10 would make the window view overlap. The
    host permutes level-3 output channels back.

v2 layout/schedule (vs the chunk-wise v1):
  - Scratch is query-major: one [128, NPOS] fp16 staging buffer per tile in
    SBUF, ONE whole-tile DMA to HBM (efficient 20KB descriptors, 12x fewer
    DMA instructions).
  - fp16 matmul inputs (PE fast-weight-load, half the prologue DMA).
  - Blends are software-pipelined `pipe` tiles behind the gathers so the
    DVE/ACT in-order queues never stall on gather completion.
  - No PE transposes: output stays [query_partition, channel] on device and
    the host transposes during assembly.

Sharding: 8 cores x 1920 queries (B*H*W = 15360 split contiguously; cores
0-3 handle batch 0, cores 4-7 batch 1). kernel() takes full inputs and
returns the full output; everything device-side runs SPMD on 8 cores.
"""

import numpy as np

import concourse.bass as bass
import concourse.mybir as mybir
import concourse.tile as tile
from concourse import bacc
from concourse.bass_utils import run_bass_kernel_spmd

# problem shape (hardcoded per harness contract)
B, D, H, W = 2, 256, 48, 160
NCORES = 8
QPC = (B * H * W) // NCORES      # 1920 queries per core
P = 128                          # queries per tile (partitions)
NT = QPC // P                    # 15 tiles per core
NLVL = 4
LH = [48, 24, 12, 6]
LW = [160, 80, 40, 20]
LHW = [LH[i] * LW[i] for i in range(NLVL)]           # 7680 1920 480 120
LOFF = [0, 7680, 9600, 10080]                        # level offset inside a query's map
NPOS = 10200
XMAJ = [True, True, True, False]                     # storage orientation
ST = [48, 24, 12, 20]                                # inner-axis size
BAND = [9 * s + 10 for s in ST]                      # 442 226 118 190
BMAX = 442
HEAD = 512                                           # scratch head/tail guard
SCRN1 = HEAD + P * NPOS + HEAD                       # per-tile scratch elems

F16 = mybir.dt.float16
F32 = mybir.dt.float32
I32 = mybir.dt.int32

PSUM_CHUNK = 1024
MM_CHUNK = 512

# per-level storage: [outer O, inner I]; levels 0-2 x-major (inner=y),
# level 3 y-major (inner=x)
INNER = [48, 24, 12, 20]
OUTER = [160, 80, 40, 6]


def _chunks(total, step):
    return [(o, min(step, total - o)) for o in range(0, total, step)]


NCPB = NCORES // B      # cores per batch
WC = W // NCPB          # w-columns per core (w-split sharding)


def _geometry(cy, tt_q):
    """Per-tile (inner_start, inner_cnt) per level, valid across all 8 cores.

    Levels 0/1 restrict the stored/computed y-rows to the union of the
    tile's query windows (coords are host-known); levels 2/3 stay full.
    With w-split sharding every core's tile t covers the same h-rows, so
    the 8-core union stays narrow. Invalid-y window taps outside the true
    map read neighbouring columns and are zeroed by the blend masks."""
    geo = []
    for t in range(NT):
        sel = tt_q == t
        ent = []
        for l in range(NLVL):
            if l >= 2:
                ent.append((0, INNER[l]))
                continue
            y0c = np.clip(np.floor(cy[sel] * (1.0 / (1 << l))), -5, LH[l] + 4)
            ys = max(int(y0c.min()) - 4, 0)
            ye = min(int(y0c.max()) + 5, LH[l] - 1)
            ent.append((ys, ye - ys + 1))
        geo.append(tuple(ent))
    return tuple(geo)


def _full_geometry():
    return tuple(tuple((0, INNER[l]) for l in range(NLVL)) for _ in range(NT))


def _xwindows(cx):
    """Per-core x-offsets + shared x-extent for levels 0/1.

    Each core's queries span a WC-wide w-slice, so its x-windows cover a
    narrow column range. The device uses ONE compile-time extent xcnt[l];
    the host packs each core's f2t slice starting at that core's xs, and
    bakes xs into the gather indices. Levels 2/3 stay full."""
    xs_all, xcnt = [], []
    for l in range(NLVL):
        if l >= 2:
            xs_all.append([0] * NCORES)
            xcnt.append(OUTER[l])
            continue
        inv = 1.0 / (1 << l)
        los, his = [], []
        for c in range(NCORES):
            x0c = np.clip(np.floor(cx[_core_ids(c)] * inv), -5, LW[l] + 4)
            los.append(max(int(x0c.min()) - 4, 0))
            his.append(min(int(x0c.max()) + 5, LW[l] - 1))
        cnt = max(h - lo + 1 for lo, h in zip(los, his))
        xs_all.append([min(lo, LW[l] - cnt) for lo in los])
        xcnt.append(cnt)
    return xs_all, xcnt


def _tile_layout(g, outer):
    """loff[l] inside a query's map, npos, band slot offsets, gather lens.

    Band slots are 10*cnt wide (gather fills 9*cnt+10 of that) so the
    blend's [10, cnt] strided view stays inside its slot."""
    loff, off = [], 0
    for l in range(NLVL):
        loff.append(off)
        off += outer[l] * g[l][1]
    bsl = [9 * g[l][1] + 10 for l in range(NLVL)]
    boff_v, o2 = [], 0
    for l in range(NLVL):
        boff_v.append(o2)
        o2 += 10 * g[l][1]
    return loff, off, boff_v, bsl


def _local_order():
    """Per-global-query local index q' = h*WC + (w % WC) and helpers."""
    g = np.arange(B * H * W)
    rr_ = g % (H * W)
    ww = rr_ % W
    qp = (rr_ // W) * WC + (ww % WC)
    return qp, (qp % P).astype(np.int64), (qp // P).astype(np.int64)


def _core_ids(c):
    """Global query ids of core c, in local q' order."""
    b, cl = c // NCPB, c % NCPB
    return (
        b * H * W
        + (np.arange(H)[:, None] * W + cl * WC + np.arange(WC)[None, :]).ravel()
    )


def _tile_chunks(g, mm_chunk, outer):
    """psum chunks: (l, cboff, csz, [(ooff, osz), ...]) — up to 2 matmul
    units of ≤mm_chunk cols each, same level, contiguous in the staging
    buffer."""
    loff, _, _, _ = _tile_layout(g, outer)
    out = []
    for l in range(NLVL):
        cnt = g[l][1]
        no = min(outer[l], max(1, mm_chunk // cnt))
        units = _chunks(outer[l], no)
        for i in range(0, len(units), 2):
            pairu = units[i : i + 2]
            csz = sum(osz * cnt for (_, osz) in pairu)
            out.append((l, loff[l] + pairu[0][0] * cnt, csz, pairu))
    return out


def build_nc(repeat=1, do_write=True, do_gather=True, do_blend=True, do_mm=True,
             do_copy=True, do_out=True, copy_pat="vaavavaavava", pipe=3, ps_bufs=4,
             f2_chunks=4, cb_bufs=2, pair=3, mm_chunk=MM_CHUNK, tt2_pool=0, gmerge=0,
             wsplit=4, geo=None):
    if geo is None:
        geo = _GEO_CACHE.get("geo") or _full_geometry()
    outer = _GEO_CACHE.get("outer") or list(OUTER)
    layouts = [_tile_layout(g, outer) for g in geo]  # (loff, npos, boff, bsl)
    tchunks = [_tile_chunks(g, mm_chunk, outer) for g in geo]
    npos_max = max(lay[1] for lay in layouts)
    btot_max = max(
        lay[2][-1] + 10 * g[-1][1] for lay, g in zip(layouts, geo)
    )
    # packed f2 layout: per-level [outer, inner] slabs
    lhw_f2 = [outer[l] * INNER[l] for l in range(NLVL)]
    loff_f2 = [sum(lhw_f2[:l]) for l in range(NLVL)]
    npos_f2 = sum(lhw_f2)
    nc = bacc.Bacc("TRN2", target_bir_lowering=False, debug=False)

    f1t = nc.dram_tensor("f1t", [2, P, QPC], F16, kind="ExternalInput")
    f2t = nc.dram_tensor("f2t", [2, P, npos_f2], F16, kind="ExternalInput")
    idxt = nc.dram_tensor("idxt", [P, NLVL * NT], I32, kind="ExternalInput")
    # stage-2 blend per-query scalars, 2 per (lvl,tile)
    wgtt = nc.dram_tensor("wgtt", [P, NLVL * NT * 2], F32, kind="ExternalInput")
    # stage-1 blend weights with validity masks folded in, 90 per (lvl,tile)
    my0t = nc.dram_tensor("my0t", [P, NLVL * NT * 90], F16, kind="ExternalInput")
    my1t = nc.dram_tensor("my1t", [P, NLVL * NT * 90], F16, kind="ExternalInput")
    outp = nc.dram_tensor("outp", [P, NT * NLVL * 81], F16, kind="ExternalOutput")

    with tile.TileContext(nc) as tc:
        with (
            tc.tile_pool(name="dram", bufs=1, space="DRAM") as dpool,
            tc.tile_pool(name="const", bufs=1) as cpool,
            tc.tile_pool(name="corrbuf", bufs=cb_bufs) as cbpool,
            tc.tile_pool(name="bands", bufs=pipe + 2) as bpool,
            tc.tile_pool(name="blend", bufs=6) as blpool,
            tc.tile_pool(name="otile", bufs=4) as opool,
            tc.tile_pool(name="psum", bufs=ps_bufs, space="PSUM") as pspool,
        ):
            # ---- constants / persistent tiles ----
            scrt = [
                dpool.tile([HEAD + P * layouts[t][1] + HEAD], F16, name=f"scrt{t}")
                for t in range(NT)
            ]

            f1sb = cpool.tile([P, 2 * QPC], F16)
            nc.sync.dma_start(f1sb[:, 0:QPC], f1t[0])
            nc.sync.dma_start(f1sb[:, QPC : 2 * QPC], f1t[1])
            f2sb0 = cpool.tile([P, npos_f2], F16)
            f2sb1 = cpool.tile([P, npos_f2], F16)
            for coff, csz in _chunks(npos_f2, -(-npos_f2 // f2_chunks)):
                nc.sync.dma_start(f2sb0[:, coff : coff + csz], f2t[0][:, coff : coff + csz])
                nc.scalar.dma_start(f2sb1[:, coff : coff + csz], f2t[1][:, coff : coff + csz])
            idx_sb = cpool.tile([P, NLVL * NT], I32)
            nc.sync.dma_start(idx_sb[:], idxt[:])
            wgt_sb = cpool.tile([P, NLVL * NT * 2], F32)
            nc.sync.dma_start(wgt_sb[:], wgtt[:])
            my0_sb = cpool.tile([P, NLVL * NT * 90], F16)
            nc.scalar.dma_start(my0_sb[:], my0t[:])
            my1_sb = cpool.tile([P, NLVL * NT * 90], F16)
            nc.scalar.dma_start(my1_sb[:], my1t[:])

            # zero head+tail guards of every scratch so gathers never read
            # uninitialized HBM (intra-tile band spill lands in written
            # neighbour sections; only the outer edges need zeroing).
            # Emitted AFTER the input loads: these 30 tiny DMAs only gate
            # tile 0's gather (~10us into the pass), while the loads gate
            # the very first matmul — queue order is launch latency.
            zguard = cpool.tile([1, HEAD], F16)
            nc.vector.memset(zguard[:], 0.0)
            for t in range(NT):
                npos_t = layouts[t][1]
                nc.sync.dma_start(scrt[t][0:HEAD].unsqueeze(0), zguard[0:1, :])
                nc.scalar.dma_start(
                    scrt[t][HEAD + P * npos_t : HEAD + P * npos_t + HEAD].unsqueeze(0),
                    zguard[0:1, :],
                )

            # [P, outer, inner] views of the two f2 halves, one per level
            f2v0 = [
                f2sb0[:, loff_f2[l] : loff_f2[l] + lhw_f2[l]].rearrange(
                    "p (o i) -> p o i", i=INNER[l]
                )
                for l in range(NLVL)
            ]
            f2v1 = [
                f2sb1[:, loff_f2[l] : loff_f2[l] + lhw_f2[l]].rearrange(
                    "p (o i) -> p o i", i=INNER[l]
                )
                for l in range(NLVL)
            ]

            def blend_stage1(t, band, l, t1):
                # t1 = g0*m0 + g1*m1 (masks fold wy blend + validity)
                s_in = geo[t][l][1]
                lo = layouts[t][2][l]
                bw = band[:, lo : lo + 10 * s_in].rearrange("p (r s) -> p r s", s=s_in)
                c90 = (l * NT + t) * 90
                m0 = my0_sb[:, c90 : c90 + 90].rearrange("p (r j) -> p r j", j=9)
                m1 = my1_sb[:, c90 : c90 + 90].rearrange("p (r j) -> p r j", j=9)
                t2 = blpool.tile([P, 90], F32, name="t2")
                t1v = t1[:].rearrange("p (r j) -> p r j", j=9)
                t2v = t2[:].rearrange("p (r j) -> p r j", j=9)
                nc.vector.tensor_tensor(
                    out=t1v, in0=bw[:, 0:10, 0:9], in1=m0, op=mybir.AluOpType.mult
                )
                tt2 = nc.gpsimd if tt2_pool else nc.vector
                tt2.tensor_tensor(
                    out=t2v, in0=bw[:, 0:10, 1:10], in1=m1, op=mybir.AluOpType.mult
                )
                nc.gpsimd.tensor_add(out=t1[:], in0=t1[:], in1=t2[:])

            def blend_stage2(t, l, t1, otile):
                # o = t1[0:9]*(1-wx) (ACT), then o = t1[1:10]*wx + o (DVE)
                t1r = t1[:].rearrange("p (r j) -> p r j", j=9)
                c2 = (l * NT + t) * 2
                ov = otile[:, l * 81 : (l + 1) * 81].rearrange("p (a j) -> p a j", j=9)
                nc.scalar.mul(ov, t1r[:, 0:9, :], wgt_sb[:, c2 : c2 + 1])
                nc.vector.scalar_tensor_tensor(
                    out=ov,
                    in0=t1r[:, 1:10, :],
                    scalar=wgt_sb[:, c2 + 1 : c2 + 2],
                    in1=ov,
                    op0=mybir.AluOpType.mult,
                    op1=mybir.AluOpType.add,
                )

            # ---- main loop over query tiles ----
            import contextlib

            rep_ctx = tc.For_i(0, repeat, 1) if repeat > 1 else contextlib.nullcontext()
            with rep_ctx:
                copy_rr = 0
                bands = {}
                blend_on = do_blend and do_gather and do_write

                def emit_tile(t, tb):
                    """tb: tile whose blend work interleaves into this tile's
                    copy stream (None for none). Level l's stage-1 is emitted
                    after chunk-group l, its stage-2 one group later, so each
                    engine sees at most a couple of small blend ops between
                    psum-draining copies and every cross-engine dependency has
                    a full group of slack."""
                    nonlocal copy_rr
                    npos_t = layouts[t][1]
                    cb = cbpool.tile([P, npos_max], F16, name="cb")
                    groups = [
                        tchunks[t][i : i + pair]
                        for i in range(0, len(tchunks[t]), pair)
                    ]
                    # index of each level's last chunk in the level-major list
                    lvl_last = {}
                    for i, (l, _, _, _) in enumerate(tchunks[t]):
                        lvl_last[l] = i
                    t1s = {}
                    otile = opool.tile([P, NLVL * 81], F16, name="otile") if tb is not None else None
                    for gi, grp in enumerate(groups):
                        pss = [
                            pspool.tile([P, PSUM_CHUNK], F32, name="cps")
                            for _ in grp
                        ]
                        for k in range(2 if do_mm else 0):
                            f2v = f2v0 if k == 0 else f2v1
                            wgt = f1sb[:, k * QPC + t * P : k * QPC + (t + 1) * P]
                            for ps, (l, cboff, csz, units) in zip(pss, grp):
                                s, cnt = geo[t][l]
                                # each unit starts on a PSUM bank boundary: a
                                # matmul's output must not cross a bank
                                for ui, (ooff, osz) in enumerate(units):
                                    nc.tensor.matmul(
                                        ps[:, ui * 512 : ui * 512 + osz * cnt],
                                        wgt,
                                        f2v[l][:, ooff : ooff + osz, s : s + cnt],
                                        start=(k == 0),
                                        stop=(k == 1),
                                        skip_group_check=(pair > 1),
                                    )
                        if do_mm and do_copy:
                            for ci, (ps, (l, cboff, csz, units)) in enumerate(
                                zip(pss, grp)
                            ):
                                s, cnt = geo[t][l]
                                loff_l = layouts[t][0][l]
                                for ui, (ooff, osz) in enumerate(units):
                                    dst = cb[
                                        :,
                                        loff_l + ooff * cnt : loff_l + (ooff + osz) * cnt,
                                    ]
                                    eng = copy_pat[copy_rr % len(copy_pat)]
                                    if eng == "v":
                                        nc.vector.tensor_copy(
                                            dst, ps[:, ui * 512 : ui * 512 + osz * cnt]
                                        )
                                    else:
                                        nc.scalar.copy(
                                            dst, ps[:, ui * 512 : ui * 512 + osz * cnt]
                                        )
                                    copy_rr += 1
                                # write scratch sections as soon as their
                                # copies are done: the transfer overlaps the
                                # remaining levels' copies, so the gathers
                                # (which wait on every write) only serialize
                                # behind the small last write. wsplit=2 cuts
                                # SP DMA-issue cost: L0 alone, then L1-3
                                # combined (contiguous in the staging buffer).
                                if do_write and gi * pair + ci == lvl_last[l]:
                                    if wsplit >= 4:
                                        a, bnd = loff_l, loff_l + outer[l] * cnt
                                    elif l == 0:
                                        a, bnd = 0, layouts[t][0][1]
                                    elif l == NLVL - 1:
                                        a, bnd = layouts[t][0][1], npos_t
                                    else:
                                        a = bnd = 0
                                    if bnd > a:
                                        nc.sync.dma_start(
                                            scrt[t][HEAD : HEAD + P * npos_t]
                                            .rearrange("(p x) -> p x", x=npos_t)[
                                                :, a:bnd
                                            ],
                                            cb[:, a:bnd],
                                        )
                        if tb is not None:
                            if gi < NLVL:
                                t1s[gi] = blpool.tile([P, 90], F32, name="t1")
                                blend_stage1(tb, bands[tb], gi, t1s[gi])
                            if 1 <= gi <= NLVL:
                                blend_stage2(tb, gi - 1, t1s[gi - 1], otile)
                    if tb is not None:
                        # small tiles have fewer chunk-groups than blend slots;
                        # emit whatever didn't fit into the interleave
                        ng = len(groups)
                        for l in range(min(ng, NLVL), NLVL):
                            t1s[l] = blpool.tile([P, 90], F32, name="t1")
                            blend_stage1(tb, bands[tb], l, t1s[l])
                        for l in range(max(min(ng - 1, NLVL), 0), NLVL):
                            blend_stage2(tb, l, t1s[l], otile)
                        bands.pop(tb)
                        if do_out:
                            nc.sync.dma_start(
                                outp[:, tb * NLVL * 81 : (tb + 1) * NLVL * 81],
                                otile[:],
                            )
                    # === 4 gathers (per-level writes already emitted) ===
                    if do_gather and do_write:
                        band = bpool.tile([P, btot_max], F16, name="band")
                        for l in range(NLVL):
                            boff_t, bsl_t = layouts[t][2], layouts[t][3]
                            nc.gpsimd.indirect_dma_start(
                                out=band[:, boff_t[l] : boff_t[l] + bsl_t[l]],
                                out_offset=None,
                                in_=scrt[t][:].unsqueeze(1),
                                in_offset=bass.IndirectOffsetOnAxis(
                                    ap=idx_sb[:, t * NLVL + l : t * NLVL + l + 1],
                                    axis=0,
                                ),
                                element_offset=0,
                            )
                        bands[t] = band

                def emit_blend_only(tb):
                    otile = opool.tile([P, NLVL * 81], F16, name="otile")
                    t1s = {}
                    for l in range(NLVL):
                        t1s[l] = blpool.tile([P, 90], F32, name="t1")
                        blend_stage1(tb, bands[tb], l, t1s[l])
                        if l >= 1:
                            blend_stage2(tb, l - 1, t1s[l - 1], otile)
                    blend_stage2(tb, NLVL - 1, t1s[NLVL - 1], otile)
                    bands.pop(tb)
                    if do_out:
                        nc.sync.dma_start(
                            outp[:, tb * NLVL * 81 : (tb + 1) * NLVL * 81], otile[:]
                        )

                for t in range(NT):
                    emit_tile(t, t - pipe if (blend_on and t >= pipe) else None)
                if blend_on:
                    for t in range(max(0, NT - pipe), NT):
                        emit_blend_only(t)

    nc.compile()
    return nc


# ---------------- host side ----------------

def _pool2(x):
    n, c, h, w = x.shape
    return x.reshape(n, c, h // 2, 2, w // 2, 2).mean(axis=(3, 5))


def _host_prep(fmap1, fmap2, coords):
    fmap1 = np.asarray(fmap1, np.float32)
    fmap2 = np.asarray(fmap2, np.float32)
    coords = np.asarray(coords, np.float32)
    scale = np.float32(1.0 / np.sqrt(D))

    # pooled fmap2 levels in [B, D, outer, inner] storage orientation, scaled
    levels = []
    cur = fmap2 * scale
    for l in range(NLVL):
        if XMAJ[l]:
            levels.append(cur.transpose(0, 1, 3, 2).astype(np.float16))  # [B,D,LW,LH]
        else:
            levels.append(cur.astype(np.float16))                        # [B,D,LH,LW]
        if l < NLVL - 1:
            cur = _pool2(cur)

    cx = coords[:, 0].reshape(-1)  # [B*H*W], query q = b*H*W + h*W + w
    cy = coords[:, 1].reshape(-1)
    nq = cx.shape[0]

    # w-split sharding: core c takes w in [cl*WC, (cl+1)*WC) of every row,
    # so tile t covers the same h-rows on every core and the per-tile
    # y-window restriction survives the 8-core union.
    _, q_tile, tt_q = _local_order()
    geo = _geometry(cy, tt_q)
    xs_all, xcnt = _xwindows(cx)
    _GEO_CACHE["geo"] = geo
    _GEO_CACHE["outer"] = xcnt
    npos_f2 = sum(xcnt[l] * INNER[l] for l in range(NLVL))
    layouts = [_tile_layout(g, xcnt) for g in geo]
    npos_q = np.array([layouts[t][1] for t in range(NT)], np.int64)[tt_q]
    g_ = np.arange(nq)
    core_q = (g_ // (H * W)) * NCPB + ((g_ % (H * W)) % W) // WC

    idx_all = np.zeros((NLVL, nq), np.int32)
    wgt_all = np.zeros((NLVL, nq, 2), np.float32)
    my0_all = np.zeros((NLVL, nq, 10, 9), np.float16)
    my1_all = np.zeros((NLVL, nq, 10, 9), np.float16)
    rr = np.arange(10)
    for l in range(NLVL):
        inv = np.float32(1.0 / (1 << l))
        x = cx * inv
        y = cy * inv
        x0 = np.floor(x)
        y0 = np.floor(y)
        wx = (x - x0).astype(np.float32)
        wy = (y - y0).astype(np.float32)
        x0c = np.clip(x0, -5, LW[l] + 4).astype(np.int64)
        y0c = np.clip(y0, -5, LH[l] + 4).astype(np.int64)
        vx = ((x0[:, None] + rr[None, :] - 4) >= 0) & (
            (x0[:, None] + rr[None, :] - 4) <= LW[l] - 1
        )  # [nq, 10] validity of x-tap x0-4+i
        vy = ((y0[:, None] + rr[None, :] - 4) >= 0) & (
            (y0[:, None] + rr[None, :] - 4) <= LH[l] - 1
        )
        s_q = np.array([geo[t][l][0] for t in range(NT)], np.int64)[tt_q]
        cnt_q = np.array([geo[t][l][1] for t in range(NT)], np.int64)[tt_q]
        loff_q = np.array([layouts[t][0][l] for t in range(NT)], np.int64)[tt_q]
        if XMAJ[l]:
            # outer = x (weight wx), inner = y (weight wy); x-offset is
            # per-core (each core's f2t slice starts at its own xs)
            xs_q = np.array(xs_all[l], np.int64)[core_q]
            idx_all[l] = (
                HEAD
                + q_tile * npos_q
                + loff_q
                + (x0c - 4 - xs_q) * cnt_q
                + (y0c - 4 - s_q)
            ).astype(np.int32)
            wgt_all[l, :, 0] = 1.0 - wx
            wgt_all[l, :, 1] = wx
            m0 = vx[:, :, None] & vy[:, None, 0:9]
            m1 = vx[:, :, None] & vy[:, None, 1:10]
            my0_all[l] = m0 * (1.0 - wy)[:, None, None]
            my1_all[l] = m1 * wy[:, None, None]
        else:
            # outer = y (weight wy), inner = x (weight wx)
            idx_all[l] = (
                HEAD + q_tile * npos_q + loff_q + (y0c - 4) * cnt_q + (x0c - 4)
            ).astype(np.int32)
            wgt_all[l, :, 0] = 1.0 - wy
            wgt_all[l, :, 1] = wy
            m0 = vy[:, :, None] & vx[:, None, 0:9]
            m1 = vy[:, :, None] & vx[:, None, 1:10]
            my0_all[l] = m0 * (1.0 - wx)[:, None, None]
            my1_all[l] = m1 * wx[:, None, None]

    f1r = fmap1.reshape(B, D, H * W).astype(np.float16)

    def core_map(c):
        b = c // NCPB
        ids = _core_ids(c)
        idl = ids - b * H * W
        f1c = f1r[b][:, idl]
        f2c = np.concatenate(
            [
                levels[l][b][:, xs_all[l][c] : xs_all[l][c] + xcnt[l]].reshape(D, -1)
                for l in range(NLVL)
            ],
            axis=1,
        )
        return {
            "f1t": np.ascontiguousarray(f1c.reshape(2, P, QPC)),
            "f2t": np.ascontiguousarray(f2c.reshape(2, P, npos_f2)),
            "idxt": np.ascontiguousarray(
                idx_all[:, ids].reshape(NLVL, NT, P).transpose(2, 1, 0).reshape(P, -1)
            ),
            "wgtt": np.ascontiguousarray(
                wgt_all[:, ids].reshape(NLVL, NT, P, 2)
                .transpose(2, 0, 1, 3)
                .reshape(P, -1)
            ),
            "my0t": np.ascontiguousarray(
                my0_all[:, ids].reshape(NLVL, NT, P, 90)
                .transpose(2, 0, 1, 3)
                .reshape(P, -1)
            ),
            "my1t": np.ascontiguousarray(
                my1_all[:, ids].reshape(NLVL, NT, P, 90)
                .transpose(2, 0, 1, 3)
                .reshape(P, -1)
            ),
        }

    return [core_map(c) for c in range(NCORES)]


def assemble(results):
    out = np.empty((B, NLVL * 81, H * W), np.float32)
    for c in range(NCORES):
        b = c // NCPB
        idl = _core_ids(c) - b * H * W
        r = np.asarray(results[c]["outp"], np.float32).reshape(P, NT, NLVL, 81)
        r = r.transpose(2, 3, 1, 0).reshape(NLVL, 81, QPC)  # local query = t*P + p
        outb = out[b]
        for l in range(NLVL):
            blk = r[l]
            if not XMAJ[l]:
                # stored channel order is bi*9+a; reference wants 9a+bi
                blk = blk.reshape(9, 9, QPC).transpose(1, 0, 2).reshape(81, QPC)
            outb[l * 81 : (l + 1) * 81, idl] = blk
    return out.reshape(B, NLVL * 81, H, W)


_NC_CACHE = {}
_GEO_CACHE = {}


def get_nc():
    key = (_GEO_CACHE.get("geo"), tuple(_GEO_CACHE.get("outer") or ()))
    if _NC_CACHE.get("key", 0) != key:
        _NC_CACHE["nc"] = build_nc()
        _NC_CACHE["key"] = key
    return _NC_CACHE["nc"]


def kernel(fmap1, fmap2, coords):
    in_maps = _host_prep(fmap1, fmap2, coords)
    nc = get_nc()
    res = run_bass_kernel_spmd(nc, in_maps, core_ids=list(range(NCORES)))
    return assemble(res.results)
